# revision 18
# baseline (speedup 1.0000x reference)
"""Trainium2 Bass kernel for the GCNPolicy net (conv1d x2 -> GCN x2 -> linear -> mean pool).

Strategy (pure data parallelism, batch sharded 8 ways):
  - per core: 4096 samples, processed in groups of 128
  - input cast fp32->bf16 during DMA (SWDGE), then 10 xbar chunk-transposes
    per group put (t', node-quad, obs) on partitions
  - conv1 as 72 small matmuls on 32x64 PE array tiles (4 node-quads run on
    separate row-quads of the PE array), conv2 on 64x64 tiles
  - GCN layers: per-node matmuls with the symmetric-normalized adjacency
    folded in as 0/1 PSUM accumulation (deg^-1/2 scales folded into ACT
    evacuation scale/bias), layer-2 mixing on GPSIMD in bf16
  - final linear + tanh + mean pool + PE transpose to restore [sample, act]
"""

import numpy as np
import ml_dtypes

import concourse.bass as bass
import concourse.mybir as mybir
from concourse.tile import TileContext

N_CORES = 8
B = 32768
BC = B // N_CORES          # samples per core
S = 128                    # samples per group
L, NN, OBS = 5, 8, 32
F32 = mybir.dt.float32
BF16 = mybir.dt.bfloat16
AF = mybir.ActivationFunctionType

# ---- graph constants (fixed 8-node graph from the model) ----
_EDGES = [(i, i + 1) for i in range(NN - 1)] + [(1, 6), (2, 5)]
_A01 = np.zeros((NN, NN), np.float32)
for _i, _j in _EDGES:
    _A01[_i, _j] = 1.0
    _A01[_j, _i] = 1.0
_A01 += np.eye(NN, dtype=np.float32)
_DD = (1.0 / np.sqrt(_A01.sum(1))).astype(np.float32)  # deg^-1/2 incl self loop

# directed neighbor pairs (i, j), i != j, grouped for PE row-tile pairing
_T0_EXTRAS_A = [(0, 1), (1, 0), (1, 2), (2, 1), (2, 3), (3, 2)]
_T0_EXTRAS_B = [(4, 3), (5, 2), (6, 1)]
_T8_EXTRAS_A = [(1, 6), (2, 5), (3, 4)]
_T8_EXTRAS_B = [(4, 5), (5, 4), (5, 6), (6, 5), (6, 7), (7, 6)]
_BASES = [0, 5, 1, 6, 2, 7, 3, 4]


_NBR = {i: [j for j in range(NN) if _A01[i, j] > 0] for i in range(NN)}


def _mix_order():
    """(i, j) list for z1+mix1 MMs: each target i's group is consecutive,
    self edge first (start=True); targets ordered to alternate PSUM banks."""
    order = []
    for i in (0, 5, 1, 6, 2, 7, 3, 4):
        js = [i] + [j for j in _NBR[i] if j != i]
        # alternate row-tiles (j//4) inside the group where possible
        js_rest = sorted(js[1:], key=lambda j: (j // 4 == i // 4, j))
        for j in [i] + js_rest:
            order.append((i, j))
    assert len(order) == 26
    return order


def build_nc(n_groups=BC // S):
    nc = bass.Bass()
    data = nc.dram_tensor("data", [BC, L * NN * OBS], F32, kind="ExternalInput")
    w1p = nc.dram_tensor("w1p", [128, 192], BF16, kind="ExternalInput")
    w2p = nc.dram_tensor("w2p", [128, 192], BF16, kind="ExternalInput")
    g1p = nc.dram_tensor("g1p", [128, 128], BF16, kind="ExternalInput")
    g2p = nc.dram_tensor("g2p", [128, 128], BF16, kind="ExternalInput")
    lwp = nc.dram_tensor("lwp", [128, 8], BF16, kind="ExternalInput")
    cb = nc.dram_tensor("cb", [128, 19], F32, kind="ExternalInput")
    idn = nc.dram_tensor("idn", [8, 8], F32, kind="ExternalInput")
    out = nc.dram_tensor("out", [BC, 8], F32, kind="ExternalOutput")

    mix_mms = _mix_order()
    # stop flag: last MM (in order) targeting slice i
    last_for = {}
    for idx, (i, _j) in enumerate(mix_mms):
        last_for[i] = idx

    with TileContext(nc) as tc:
        with tc.tile_pool(name="wpool", bufs=1) as wp, \
             tc.tile_pool(name="io", bufs=2) as io, \
             tc.tile_pool(name="act", bufs=2) as ap, \
             tc.tile_pool(name="psum", bufs=1, space="PSUM") as pp:

            w1sb = wp.tile([128, 192], BF16, name="w1sb")
            nc.sync.dma_start(out=w1sb, in_=w1p.ap())
            w2sb = wp.tile([128, 192], BF16, name="w2sb")
            nc.sync.dma_start(out=w2sb, in_=w2p.ap())
            g1sb = wp.tile([128, 128], BF16, name="g1sb")
            nc.sync.dma_start(out=g1sb, in_=g1p.ap())
            g2sb = wp.tile([128, 128], BF16, name="g2sb")
            nc.sync.dma_start(out=g2sb, in_=g2p.ap())
            lwsb = wp.tile([128, 8], BF16, name="lwsb")
            nc.sync.dma_start(out=lwsb, in_=lwp.ap())
            cbsb = wp.tile([128, 19], F32, name="cbsb")
            nc.sync.dma_start(out=cbsb, in_=cb.ap())
            idsb = wp.tile([8, 8], F32, name="idsb")
            nc.sync.dma_start(out=idsb, in_=idn.ap())

            for g in range(n_groups):
                row0 = g * S

                # ---- load + cast + transpose ----
                xg = io.tile([128, 1280], BF16, tag="xg")
                nc.gpsimd.dma_start(out=xg, in_=data.ap()[row0:row0 + S, :])
                xT = io.tile([128, 1280], BF16, tag="xT")
                for c in range(10):
                    nc.sync.dma_start(
                        out=xT[:, 128 * c:128 * (c + 1)],
                        in_=xg[:, 128 * c:128 * (c + 1)],
                        transpose=True,
                    )

                # ---- conv1: y1P[q][64*nh + o, 128*t + s] = y1[o,t,n=nh*4+q,s]
                y1P = [pp.tile([128, 384], F32, tag=f"y1_{q}", name=f"y1P{q}") for q in range(4)]
                for t in range(3):
                    for nh in range(2):
                        for k in range(3):
                            c = 2 * (t + k) + nh
                            for q in range(4):
                                nc.tensor.matmul(
                                    y1P[q][64 * nh:64 * nh + 64, 128 * t:128 * t + 128],
                                    w1sb[32 * q:32 * q + 32, 64 * k:64 * k + 64],
                                    xT[32 * q:32 * q + 32, 128 * c:128 * c + 128],
                                    start=(k == 0), stop=(k == 2),
                                    tile_position=(32 * q, 64 * nh),
                                )
                y1sb = [ap.tile([128, 384], BF16, tag=f"y1sb_{q}", name=f"y1sb{q}") for q in range(4)]
                for q in range(4):
                    nc.scalar.activation(y1sb[q], y1P[q], AF.Relu,
                                         bias=cbsb[:, 0:1], scale=1.0)

                # ---- conv2: y2ps[0:64, 128*n + s] = y2[u, n, s]
                # row tile r = n//4 reads y1 partition half; all outputs land
                # on psum partitions 0-63 (accumulation stays within one
                # tile_position; T0 hits bank of slices 0-3, T8 of 4-7)
                y2ps = pp.tile([64, 1024], F32, tag="y2ps")
                for step in range(4):
                    for n in (step, 4 + step):
                        r, m = n // 4, n % 4
                        for k in range(3):
                            nc.tensor.matmul(
                                y2ps[0:64, 128 * n:128 * n + 128],
                                w2sb[64 * r:64 * r + 64, 64 * k:64 * k + 64],
                                y1sb[m][64 * r:64 * r + 64, 128 * k:128 * k + 128],
                                start=(k == 0), stop=(k == 2),
                                tile_position=(64 * r, 0),
                            )
                # evac with d_j scale folded: y2d = relu(d_j * conv2 + d_j*b2)
                y2sb = ap.tile([64, 1024], BF16, tag="y2sb")
                for n in range(8):
                    nc.scalar.activation(
                        y2sb[:, 128 * n:128 * n + 128],
                        y2ps[0:64, 128 * n:128 * n + 128],
                        AF.Relu, bias=cbsb[0:64, 1 + n:2 + n],
                        scale=float(_DD[n]))

                # ---- z1 + mix1 on PE (0/1 adjacency PSUM accumulation) ----
                zm1 = pp.tile([128, 1024], F32, tag="zm")
                for idx, (i, j) in enumerate(mix_mms):
                    nc.tensor.matmul(
                        zm1[:, 128 * i:128 * i + 128],
                        g1sb[0:64, :],
                        y2sb[:, 128 * j:128 * j + 128],
                        start=(i == j), stop=(last_for[i] == idx),
                    )
                # h1d = relu(d_i^2 * m1 + d_i*gb1)   (= d_i * h1, pre-scaled for L2)
                h1sb = ap.tile([128, 1024], BF16, tag="h1sb")
                for i in range(8):
                    nc.scalar.activation(
                        h1sb[:, 128 * i:128 * i + 128], zm1[:, 128 * i:128 * i + 128],
                        AF.Relu, bias=cbsb[:, 9 + i:10 + i], scale=float(_DD[i] ** 2))

                # ---- z2 (K split into two 64-halves on PE row tiles) ----
                z2p = pp.tile([128, 1024], F32, tag="zm")
                for j in range(8):
                    nc.tensor.matmul(
                        z2p[:, 128 * j:128 * j + 128], g2sb,
                        h1sb[:, 128 * j:128 * j + 128],
                        start=True, stop=True)

                z2sb = ap.tile([128, 1024], BF16, tag="z2sb")
                nc.vector.tensor_copy(out=z2sb, in_=z2p)

                # ---- mix2 on GPSIMD (bf16, strided shift-adds) ----
                a2 = ap.tile([128, 896], BF16, tag="a2")
                m2sb = ap.tile([128, 1024], BF16, tag="m2sb")
                nc.gpsimd.tensor_add(a2, z2sb[:, 0:896], z2sb[:, 128:1024])
                nc.gpsimd.tensor_add(m2sb[:, 128:896], a2[:, 0:768], z2sb[:, 256:1024])
                nc.gpsimd.tensor_copy(out=m2sb[:, 0:128], in_=a2[:, 0:128])
                nc.gpsimd.tensor_copy(out=m2sb[:, 896:1024], in_=a2[:, 768:896])
                for (i, j) in [(1, 6), (6, 1), (2, 5), (5, 2)]:
                    nc.gpsimd.tensor_add(
                        m2sb[:, 128 * i:128 * i + 128],
                        m2sb[:, 128 * i:128 * i + 128],
                        z2sb[:, 128 * j:128 * j + 128])

                # h2 = relu(d_i * m2 + gb2)
                h2sb = ap.tile([128, 1024], BF16, tag="h2sb")
                for i in range(8):
                    nc.scalar.activation(
                        h2sb[:, 128 * i:128 * i + 128], m2sb[:, 128 * i:128 * i + 128],
                        AF.Relu, bias=cbsb[:, 17:18], scale=float(_DD[i]))

                # ---- lw + tanh + pool + transpose out ----
                o_ps = pp.tile([128, 1024], F32, tag="zm")
                for h in range(2):
                    nc.tensor.matmul(o_ps[0:8, 512 * h:512 * h + 512], lwsb,
                                     h2sb[:, 512 * h:512 * h + 512],
                                     start=True, stop=True)
                o_sb = ap.tile([8, 1024], F32, tag="o_sb")
                nc.scalar.activation(o_sb, o_ps[0:8, :], AF.Tanh,
                                     bias=cb_lb(cbsb), scale=1.0)
                t1 = ap.tile([8, 512], F32, tag="t1")
                nc.vector.tensor_add(t1, o_sb[:, 0:512], o_sb[:, 512:1024])
                t2 = ap.tile([8, 256], F32, tag="t2")
                nc.vector.tensor_add(t2, t1[:, 0:256], t1[:, 256:512])
                t3 = ap.tile([8, 128], F32, tag="t3")
                nc.vector.tensor_add(t3, t2[:, 0:128], t2[:, 128:256])

                oT = pp.tile([128, 8], F32, tag="y1_0")
                nc.tensor.transpose(oT, t3, idsb)
                oF = ap.tile([128, 8], F32, tag="oF")
                nc.scalar.activation(oF, oT, AF.Copy, bias=0.0, scale=0.125)
                nc.sync.dma_start(out=out.ap()[row0:row0 + S, :], in_=oF)

    from drainfix import split_multiwait_ctrl
    split_multiwait_ctrl(nc)
    return nc


def cb_lb(cbsb):
    return cbsb[0:8, 18:19]


def pack_inputs(w1, b1, w2, b2, gw1, gb1, gw2, gb2, lw, lb):
    """Host-side packing of weights/constants into device layouts."""
    bf = ml_dtypes.bfloat16
    w1p = np.zeros((128, 192), bf)
    for q in range(4):
        # w1p[q*32+i, k*64+o] = w1[o, i, k]
        w1p[32 * q:32 * q + 32] = np.transpose(w1, (1, 2, 0)).reshape(32, 192).astype(bf)
    # careful: need [i, (k, o)] ordering: transpose(w1,(1,2,0)) gives [i, k, o] ✓
    w2p = np.zeros((128, 192), bf)
    w2ko = np.transpose(w2, (1, 2, 0)).reshape(64, 192)  # [o, (k, u)]
    for r in range(2):
        w2p[64 * r:64 * r + 64] = w2ko.astype(bf)
    g1p = np.zeros((128, 128), bf)
    g1p[0:64] = gw1.astype(bf)
    g1p[64:128] = gw1.astype(bf)
    g2p = gw2.astype(bf)
    lwp = lw.astype(bf)

    cbm = np.zeros((128, 19), np.float32)
    cbm[:, 0] = np.concatenate([b1, b1])
    for n in range(8):
        cbm[0:64, 1 + n] = _DD[n] * b2
    for i in range(8):
        cbm[:, 9 + i] = _DD[i] * gb1
    cbm[:, 17] = gb2
    cbm[0:8, 18] = lb
    idn = np.eye(8, dtype=np.float32)
    return dict(w1p=w1p, w2p=w2p, g1p=g1p, g2p=g2p, lwp=lwp, cb=cbm, idn=idn)


_RUNNER = {}


def _get_runner():
    """Build the Bass module once and wrap it in a cached jitted shard_map
    (adapted from concourse.bass2jax.run_bass_via_pjrt so repeat calls reuse
    the compiled executable)."""
    if "fn" in _RUNNER:
        return _RUNNER["fn"]
    import jax
    from jax.sharding import Mesh, PartitionSpec
    from jax.experimental.shard_map import shard_map
    from concourse import bass2jax
    from concourse.bass2jax import _bass_exec_p, install_neuronx_cc_hook
    import concourse.mybir as mb

    install_neuronx_cc_hook()
    nc = build_nc()

    part_name = nc.partition_id_tensor.name if nc.partition_id_tensor else None
    in_names, out_names, out_avals, zero_outs = [], [], [], []
    for alloc in nc.m.functions[0].allocations:
        if not isinstance(alloc, mb.MemoryLocationSet):
            continue
        name = alloc.memorylocations[0].name
        if alloc.kind == "ExternalInput":
            if name != part_name:
                in_names.append(name)
        elif alloc.kind == "ExternalOutput":
            out_names.append(name)
            shape = tuple(alloc.tensor_shape)
            dtype = mb.dt.np(alloc.dtype)
            out_avals.append(jax.core.ShapedArray(shape, dtype))
            zero_outs.append(np.zeros(shape, dtype))
    n_params = len(in_names)
    all_in = in_names + out_names + ([part_name] if part_name else [])

    def _body(*args):
        operands = list(args)
        if part_name is not None:
            operands.append(bass2jax.partition_id_tensor())
        outs = _bass_exec_p.bind(
            *operands, out_avals=tuple(out_avals), in_names=tuple(all_in),
            out_names=tuple(out_names), lowering_input_output_aliases=(),
            sim_require_finite=True, sim_require_nnan=True, nc=nc)
        return tuple(outs)

    devices = jax.devices()[:N_CORES]
    mesh = Mesh(np.asarray(devices), ("core",))
    n_outs = len(out_names)
    sharded = jax.jit(
        shard_map(_body, mesh=mesh,
                  in_specs=(PartitionSpec("core"),) * (n_params + n_outs),
                  out_specs=(PartitionSpec("core"),) * n_outs,
                  check_rep=False),
        donate_argnums=tuple(range(n_params, n_params + n_outs)),
        keep_unused=True)

    _RUNNER["fn"] = (sharded, in_names, out_names, zero_outs)
    return _RUNNER["fn"]


def _make_args(data, packed):
    sharded, in_names, out_names, zero_outs = _get_runner()
    per_core = []
    for c in range(N_CORES):
        m = {"data": data[c * BC:(c + 1) * BC]}
        m.update(packed)
        per_core.append([np.asarray(m[k]) for k in in_names])
    concat_in = [np.concatenate([per_core[c][i] for c in range(N_CORES)], axis=0)
                 for i in range(len(in_names))]
    concat_zeros = [np.zeros((N_CORES * z.shape[0], *z.shape[1:]), z.dtype)
                    for z in zero_outs]
    return concat_in, concat_zeros


def run_packed(data, packed):
    sharded, in_names, out_names, zero_outs = _get_runner()
    concat_in, concat_zeros = _make_args(data, packed)
    out_arrs = sharded(*concat_in, *concat_zeros)
    return np.asarray(out_arrs[out_names.index("out")])


def kernel(data, w1, b1, w2, b2, gw1, gb1, gw2, gb2, lw, lb):
    data = np.ascontiguousarray(np.asarray(data, dtype=np.float32)).reshape(B, -1)
    packed = pack_inputs(np.asarray(w1), np.asarray(b1), np.asarray(w2),
                         np.asarray(b2), np.asarray(gw1), np.asarray(gb1),
                         np.asarray(gw2), np.asarray(gb2), np.asarray(lw),
                         np.asarray(lb))
    return run_packed(data, packed)
